# revision 37
# baseline (speedup 1.0000x reference)
"""Complex LSTM cell (CLSTMCell) Trainium2 kernel — fp8 DoubleRow edition.

Full inputs in, full outputs out. Data-parallel over batch: B=4096 rows
sharded 512/core across 8 NeuronCores; weights replicated (host pre-packed).

Math: with X1=[xr|hr], X2=[xi|hi] ([B,2048]) and W1=[Ur;Wr], W2=[Ui;Wi]
([2048,4096]), gate projections run on the PE in fp8-e4m3 DoubleRow mode
(2 contraction rows per cycle). Each fp32 operand A is represented as
fp8 pair A = Ahi + Alo (Alo = fp8 residual, exact in e4m3's wide exponent
range), recovering ~bf16 accuracy from fp8 matmuls at 3 chains/product:
  A@B ~= Ahi@Bhi + Alo@Bhi + Ahi@Blo.

Gates a, o (most error-sensitive: tanh, and h_t's outer product) use
complex Karatsuba (P1=X1W1, P2=X2W2, P3=X3W3 with X3=X1+X2, W3=W1+W2;
Zr=P1-P2, Zi=P3-P1-P2) with fully split products (9 chains each).
Gate f uses the direct form with X-side split only (8 chains) and gate i
(least sensitive) the direct form with hi-only products (4 chains), both
PSUM-accumulated via negated X2 copies, e.g.
  Zr_i = X1hi@W1hi + (-X2hi)@W2hi,  Zi_i = X1hi@W2hi + X2hi@W1hi.
Measured rel_l2 vs fp32 reference: h_t 1.48e-2, c_t 1.39e-2 (gate 2e-2).

Matmuls are oriented weights-stationary: PSUM tiles are [128 o-cols,
512 batch], so the gate bias is a per-partition scalar folded into the
Activation op's scale+bias, and the cell-update epilogue runs in fp16 on
DVE at 2x throughput. Outputs return transposed [2H, B/core] in fp16 and
are unpacked on host.
"""

import sys

for _p in ("/opt/trn_rl_repo",):
    if _p not in sys.path:
        sys.path.insert(0, _p)

import numpy as np
import ml_dtypes

import concourse.bass as bass
import concourse.mybir as mybir
from concourse.bass_utils import run_bass_kernel_spmd
from concourse.tile import TileContext

F32 = mybir.dt.float32
F16 = mybir.dt.float16
FP8 = mybir.dt.float8e4
NPFP8 = ml_dtypes.float8_e4m3  # TRN e4m3: max normal 240
AFT = mybir.ActivationFunctionType
DR = mybir.MatmulPerfMode.DoubleRow

B = 4096
IN = 1024
H = 1024
NCORES = 8
BL = B // NCORES          # 512 batch rows per core (= max moving free)
K = 2 * IN                # 2048 contraction
KT = K // 128             # 16 k-tiles
OB = H // 8               # -
NOB = 8                   # o-blocks of 128
SX, SWT = 16.0, 256.0     # fp8 quantization scales
SINV = 1.0 / (SX * SWT)

# X slab order in dram/SBUF (i/f-gate consumers first so their chains
# can start before the karatsuba slabs land)
XS = ("x1h", "x2nh", "x2h", "x1l", "x2nl", "x2l", "x3h", "x3l")
# Per-gate form: i = direct hi-only (4 chains), f = direct with X hi+lo
# split (8 chains), a/o = karatsuba with full hi/lo splits (9 chains).
# Gate column order in the packed weight tensor: f, i, a, o (ref order).
KAR_GATES = (2, 3)        # a, o
WS = [(1, "1h"), (1, "2h"), (0, "1h"), (0, "2h")]
for _g in KAR_GATES:
    for _s in ("1h", "1l", "2h", "2l", "3h", "3l"):
        WS.append((_g, _s))
NSLAB = len(WS)           # 16


def _split_multiwait_json(raw: bytes) -> bytes:
    """The walrus build in this container accepts at most one sem wait
    per instruction; Tile's scheduler packs several. Split the extras
    into preceding wait-only EventSemaphore instructions on the same
    engine (same semantics: the sequencer blocks on each in order)."""
    import orjson

    m = orjson.loads(raw)
    ctr = 0
    for fn in m["functions"]:
        for bb in fn["blocks"]:
            out = []
            for ins in bb["instructions"]:
                si = ins.get("sync_info")
                waits = si.get("on_wait") if si else None
                if waits and len(waits) > 1:
                    for w in waits[:-1]:
                        ctr += 1
                        nop = {
                            "engine": ins["engine"],
                            "ins": [],
                            "outs": [],
                            "name": f"{ins['name']}_sw{ctr}",
                            "opcode": "EventSemaphore",
                            "sync_info": {"on_update": [], "on_wait": [w]},
                        }
                        if "debug" in ins:
                            nop["debug"] = ins["debug"]
                        out.append(nop)
                    si["on_wait"] = [waits[-1]]
                out.append(ins)
            bb["instructions"] = out
    return orjson.dumps(m)


def _build_program():
    nc = bass.Bass()

    xpk = nc.dram_tensor("xpk", [len(XS), 128, KT, BL], FP8, kind="ExternalInput")
    wpk = nc.dram_tensor("wpk", [NOB, NSLAB, 128, KT * 128], FP8, kind="ExternalInput")
    ctp = nc.dram_tensor("ctp", [128, 16, BL], F16, kind="ExternalInput")
    bpk = nc.dram_tensor("bpk", [128, 64], F32, kind="ExternalInput")
    ht_out = nc.dram_tensor("ht_out", [16, 128, BL], F16, kind="ExternalOutput")
    ct_out = nc.dram_tensor("ct_out", [16, 128, BL], F16, kind="ExternalOutput")

    with TileContext(nc) as tc:
        with (
            tc.tile_pool(name="xc", bufs=1) as xcp,
            tc.tile_pool(name="w", bufs=2) as wp,
            tc.tile_pool(name="z", bufs=2) as zp,
            tc.tile_pool(name="g", bufs=2) as gp,
            tc.tile_pool(name="cell", bufs=1) as cp,
            tc.tile_pool(name="ps", bufs=8, space="PSUM") as pp,
        ):
            # The cost model runs each DMA queue's transfers concurrently
            # (SP + Activation HWDGE, Pool SWDGE): round-robin every input
            # DMA across all three, in first-use order.
            _queues = (nc.sync, nc.scalar, nc.gpsimd)
            _qi = [0]

            def qrr():
                q = _queues[_qi[0] % 3]
                _qi[0] += 1
                return q

            xt = {}
            for si, sname in enumerate(XS):
                xt[sname] = xcp.tile(
                    [128, KT, BL], FP8, tag=f"x_{sname}", name=f"x_{sname}"
                )

            def dma_x(sname, split=1):
                si = XS.index(sname)
                t = xt[sname]
                kq = KT // split
                for s in range(split):
                    qrr().dma_start(
                        out=t[:, s * kq : (s + 1) * kq, :],
                        in_=xpk[si, :, s * kq : (s + 1) * kq, :],
                    )

            # ct/bias tiles: DMAs deferred into the ob0 stream (first use is
            # the epilogue ~5us in; ct is bulky and must not block the first
            # weight slabs at the head of the SWDGE queue)
            ct = xcp.tile([128, 16, BL], F16, tag="ct", name="ct")
            bias = xcp.tile([128, 64], F32, tag="bias", name="bias")

            def chain_mm(ps, wslab, xnames, first, last, msl=slice(0, BL)):
                # accumulate sum_k W[k].T @ X[k] (DoubleRow k-pairs) for one
                # or more (wslab, xslab) products into psum tile ps
                n = len(xnames)
                for ci, xn in enumerate(xnames):
                    xs = xt[xn]
                    for kp in range(KT // 2):
                        nc.tensor.matmul(
                            ps[:, msl],
                            lhsT=wslab[ci][:, 2 * kp : 2 * kp + 2, :],
                            rhs=xs[:, 2 * kp : 2 * kp + 2, msl],
                            start=(first and ci == 0 and kp == 0),
                            stop=(last and ci == n - 1 and kp == KT // 2 - 1),
                            perf_mode=DR,
                        )

            for ob in range(NOB):
                # weight slabs for this o-block (one DMA per slab kind, on
                # the otherwise-idle SP queue)
                ws = {}

                def dma_w(g, sname):
                    si = WS.index((g, sname))
                    t = wp.tile(
                        [128, KT, 128], FP8, tag=f"w{si}", name=f"w_{g}_{sname}"
                    )
                    qrr().dma_start(
                        out=t[:],
                        in_=wpk[ob, si].rearrange("p (kt o) -> p kt o", kt=KT),
                    )
                    ws[(g, sname)] = t

                if ob == 0:
                    # interleave X-slab loads with ob0 weight slabs in exact
                    # first-use order so the PE starts right away
                    dma_x("x1h", split=2)
                    dma_w(1, "1h")
                    dma_w(1, "2h")
                    dma_x("x2nh")
                    dma_x("x2h")
                    dma_w(0, "1h")
                    dma_w(0, "2h")
                    dma_x("x1l")
                    dma_x("x2nl")
                    dma_x("x2l")
                    dma_x("x3h")
                    dma_x("x3l")
                    nc.gpsimd.dma_start(out=bias[:], in_=bpk[:])
                    for g in (2, 3):
                        for s in ("1h", "1l", "2h", "2l", "3h", "3l"):
                            dma_w(g, s)
                    nc.gpsimd.dma_start(out=ct[:], in_=ctp[:])
                else:
                    for g, sname in WS:
                        dma_w(g, sname)

                gates = {}  # (gate, 'r'/'i') -> fp16 [128, BL]

                def gate_tile(g, part):
                    out = gp.tile(
                        [128, BL], F16, tag=f"g{g}{part}", name=f"g{g}{part}_{ob}"
                    )
                    gates[(g, part)] = out
                    return out

                def act_gate(g, part, src, func, out=None, sl=slice(0, BL)):
                    bcol = g * 16 + ob * 2 + (0 if part == "r" else 1)
                    if out is None:
                        out = gate_tile(g, part)
                    nc.scalar.activation(
                        out[:, sl], src[:, sl], func,
                        bias=bias[:, bcol : bcol + 1], scale=SINV,
                    )

                last_ob = ob == NOB - 1

                def emit_i():
                    # direct, hi-only, PSUM-accumulated; acts read PSUM
                    zri = pp.tile([128, BL], F32, tag="ps", name=f"zri_{ob}")
                    chain_mm(zri, [ws[(1, "1h")], ws[(1, "2h")]],
                             ["x1h", "x2nh"], True, True)
                    act_gate(1, "r", zri, AFT.Sigmoid)
                    zii = pp.tile([128, BL], F32, tag="ps", name=f"zii_{ob}")
                    chain_mm(zii, [ws[(1, "2h")], ws[(1, "1h")]],
                             ["x1h", "x2h"], True, True)
                    act_gate(1, "i", zii, AFT.Sigmoid)

                def emit_f():
                    # direct, X hi+lo split, PSUM-accumulated; acts read PSUM.
                    # On the last o-block f is emitted last: chunk its acts so
                    # the cell update drains in halves (shorter tail).
                    w1f, w2f = ws[(0, "1h")], ws[(0, "2h")]
                    zrf = pp.tile([128, BL], F32, tag="ps", name=f"zrf_{ob}")
                    chain_mm(zrf, [w1f, w1f, w2f, w2f],
                             ["x1h", "x1l", "x2nh", "x2nl"], True, True)
                    zif = pp.tile([128, BL], F32, tag="ps", name=f"zif_{ob}")
                    chain_mm(zif, [w2f, w2f, w1f, w1f],
                             ["x1h", "x1l", "x2h", "x2l"], True, True)
                    gr = gate_tile(0, "r")
                    gi = gate_tile(0, "i")
                    for sl in chunks:
                        act_gate(0, "r", zrf, AFT.Sigmoid, out=gr, sl=sl)
                        act_gate(0, "i", zif, AFT.Sigmoid, out=gi, sl=sl)

                def emit_kar(g, kchunks):
                    func = AFT.Tanh if g == 2 else AFT.Sigmoid
                    P = []
                    for p, xb in ((1, "x1"), (2, "x2"), (3, "x3")):
                        ps = pp.tile([128, BL], F32, tag="ps",
                                     name=f"p{p}_{g}_{ob}")
                        wl = [ws[(g, f"{p}h")], ws[(g, f"{p}h")],
                              ws[(g, f"{p}l")]]
                        xl = [f"{xb}h", f"{xb}l", f"{xb}h"]
                        if p == 3 and len(kchunks) > 1:
                            # last-drained product: separate accumulation
                            # group per batch half so the epilogue's first
                            # chunk starts before the second half's chains end
                            for msl in kchunks:
                                chain_mm(ps, wl, xl, True, True, msl=msl)
                        else:
                            chain_mm(ps, wl, xl, True, True)
                        P.append(ps)
                    p1, p2, p3 = P
                    # DVE may read only one PSUM operand per op: stage P2
                    # into SBUF via the Activation engine first.
                    p2s = zp.tile([128, BL], F32, tag="p2s", name=f"p2s_{g}_{ob}")
                    zr = zp.tile([128, BL], F32, tag="zr", name=f"zr_{g}_{ob}")
                    q = zp.tile([128, BL], F32, tag="q", name=f"q_{g}_{ob}")
                    zi = zp.tile([128, BL], F32, tag="zi", name=f"zi_{g}_{ob}")
                    gr = gate_tile(g, "r")
                    gi = gate_tile(g, "i")
                    # stage the P2 copies and P1+-P2 combines for all chunks
                    # first: they depend only on P1/P2 (done well before P3)
                    # and must not queue behind chunk-1 acts on Act
                    for sl in kchunks:
                        nc.scalar.copy(p2s[:, sl], p2[:, sl])
                        nc.vector.tensor_sub(zr[:, sl], p1[:, sl], p2s[:, sl])
                        nc.vector.tensor_add(q[:, sl], p1[:, sl], p2s[:, sl])
                    for sl in kchunks:
                        nc.vector.tensor_sub(zi[:, sl], p3[:, sl], q[:, sl])
                        act_gate(g, "r", zr, func, out=gr, sl=sl)
                        act_gate(g, "i", zi, func, out=gi, sl=sl)

                # last o-block: chunk the o-gate recombine and the final
                # h = o*tanh(ct) product so the pipeline drains in pieces
                full = [slice(0, BL)]
                halves = [slice(0, BL // 2), slice(BL // 2, BL)]
                chunks = full
                emit_i()
                emit_f()
                emit_kar(2, full)
                emit_kar(3, halves if last_ob else full)
                chunks = halves if last_ob else full

                # ---- cell update (fp16 on DVE) ----
                def tmp(tag):
                    return cp.tile([128, BL], F16, tag=tag, name=f"{tag}_{ob}")

                fr, fi = gates[(0, "r")], gates[(0, "i")]
                ir_, ii_ = gates[(1, "r")], gates[(1, "i")]
                ar, ai = gates[(2, "r")], gates[(2, "i")]
                orr, oi = gates[(3, "r")], gates[(3, "i")]

                u1, u2, u3, u4 = tmp("u1"), tmp("u2"), tmp("u3"), tmp("u4")
                v1, v2, v3, v4 = tmp("v1"), tmp("v2"), tmp("v3"), tmp("v4")
                cfr, cfi = tmp("cfr"), tmp("cfi")
                air, aii = tmp("air"), tmp("aii")
                ctr, cti = tmp("ctr"), tmp("cti")
                trr, tri = tmp("trr"), tmp("tri")
                w1, w2, w3, w4 = tmp("w1"), tmp("w2"), tmp("w3"), tmp("w4")
                htr, hti = tmp("htr"), tmp("hti")
                for sl in chunks:
                    cr = ct[:, ob, sl]
                    ci = ct[:, 8 + ob, sl]
                    nc.vector.tensor_mul(u1[:, sl], cr, fr[:, sl])
                    nc.vector.tensor_mul(u2[:, sl], ci, fi[:, sl])
                    nc.vector.tensor_mul(u3[:, sl], cr, fi[:, sl])
                    nc.vector.tensor_mul(u4[:, sl], ci, fr[:, sl])
                    nc.vector.tensor_mul(v1[:, sl], ar[:, sl], ir_[:, sl])
                    nc.vector.tensor_mul(v2[:, sl], ai[:, sl], ii_[:, sl])
                    nc.vector.tensor_mul(v3[:, sl], ar[:, sl], ii_[:, sl])
                    nc.vector.tensor_mul(v4[:, sl], ai[:, sl], ir_[:, sl])
                    nc.vector.tensor_sub(cfr[:, sl], u1[:, sl], u2[:, sl])
                    nc.vector.tensor_add(cfi[:, sl], u3[:, sl], u4[:, sl])
                    nc.vector.tensor_sub(air[:, sl], v1[:, sl], v2[:, sl])
                    nc.vector.tensor_add(aii[:, sl], v3[:, sl], v4[:, sl])
                    nc.vector.tensor_add(ctr[:, sl], cfr[:, sl], air[:, sl])
                    nc.vector.tensor_add(cti[:, sl], cfi[:, sl], aii[:, sl])
                    if not last_ob:
                        nc.gpsimd.dma_start(out=ct_out[ob][:, sl], in_=ctr[:, sl])
                        nc.gpsimd.dma_start(out=ct_out[8 + ob][:, sl],
                                            in_=cti[:, sl])
                    nc.scalar.activation(trr[:, sl], ctr[:, sl], AFT.Tanh)
                    nc.scalar.activation(tri[:, sl], cti[:, sl], AFT.Tanh)
                    # on the last o-block the h=o*tanh(ct) product is the
                    # drain path: Pool takes the first chunk's imag half
                    # (starts earliest, Pool ops are ~2.6x slower than DVE)
                    veng = nc.vector
                    ieng = nc.gpsimd if last_ob else nc.vector
                    veng.tensor_mul(w1[:, sl], orr[:, sl], trr[:, sl])
                    veng.tensor_mul(w2[:, sl], oi[:, sl], tri[:, sl])
                    ieng.tensor_mul(w3[:, sl], orr[:, sl], tri[:, sl])
                    ieng.tensor_mul(w4[:, sl], oi[:, sl], trr[:, sl])
                    veng.tensor_sub(htr[:, sl], w1[:, sl], w2[:, sl])
                    ieng.tensor_add(hti[:, sl], w3[:, sl], w4[:, sl])
                    if not last_ob:
                        # final stores ride separate queues so they drain in
                        # parallel at the end of the kernel
                        nc.sync.dma_start(out=ht_out[ob][:, sl], in_=htr[:, sl])
                        nc.scalar.dma_start(out=ht_out[8 + ob][:, sl],
                                            in_=hti[:, sl])
                if last_ob:
                    # drain: full-width stores, real parts on SP / imag on
                    # Act, dispatched in data-readiness order (ct first)
                    nc.sync.dma_start(out=ct_out[ob], in_=ctr[:])
                    nc.scalar.dma_start(out=ct_out[8 + ob], in_=cti[:])
                    nc.sync.dma_start(out=ht_out[ob], in_=htr[:])
                    nc.scalar.dma_start(out=ht_out[8 + ob], in_=hti[:])
    return nc


_NC_CACHE = None


def _get_program():
    global _NC_CACHE
    if _NC_CACHE is None:
        nc = _build_program()
        fixed = _split_multiwait_json(nc.to_json_bytes())
        nc.to_json_bytes = lambda: fixed
        _NC_CACHE = nc
    return _NC_CACHE


def _q8(a):
    return np.clip(a, -240.0, 240.0).astype(NPFP8)


def _split8(a):
    hi = _q8(a)
    lo = _q8(a - hi.astype(np.float32))
    return hi, lo


def _pack_x(x1, x2):
    # x1, x2: [BL, 2048] fp32 (pre-scaled). returns [8, 128, KT, BL] fp8
    out = np.empty((len(XS), 128, KT, BL), NPFP8)
    x3 = x1 + x2
    h1, l1 = _split8(x1)
    h2, l2 = _split8(x2)
    h3, l3 = _split8(x3)
    n2 = (-h2.astype(np.float32)).astype(NPFP8)
    nl2 = (-l2.astype(np.float32)).astype(NPFP8)
    for si, arr in enumerate((h1, n2, h2, l1, nl2, l2, h3, l3)):
        # [BL, K] -> X.T k-tiles [128, KT, BL]
        out[si] = arr.T.reshape(KT, 128, BL).transpose(1, 0, 2)
    return out


def _pack_w(W1, W2):
    # W1, W2: [2048, 4H] fp32 (pre-scaled), gate-major columns [f,i,a,o].
    # returns [NOB, NSLAB, 128, KT*128] fp8
    W3 = W1 + W2
    spl = {}
    for p, W in ((1, W1), (2, W2), (3, W3)):
        spl[f"{p}h"], spl[f"{p}l"] = _split8(W)
    out = np.empty((NOB, NSLAB, 128, KT * 128), NPFP8)
    for si, (g, sname) in enumerate(WS):
        Wg = spl[sname][:, g * H : (g + 1) * H]  # [2048, 1024]
        for ob in range(NOB):
            blk = Wg[:, ob * 128 : (ob + 1) * 128]  # [2048, 128]
            out[ob, si] = (
                blk.reshape(KT, 128, 128).transpose(1, 0, 2).reshape(128, KT * 128)
            )
    return out


def kernel(input, h_x, c_x, Uw_r, Uw_i, Ub_r, Ub_i, Ww_r, Ww_i, Wb_r, Wb_i,
           _trace=False):
    input = np.asarray(input, dtype=np.float32)
    h_x = np.asarray(h_x, dtype=np.float32)
    c_x = np.asarray(c_x, dtype=np.float32)

    W1 = np.concatenate(
        [np.transpose(np.asarray(Uw_r, np.float32), (2, 0, 1)),
         np.transpose(np.asarray(Ww_r, np.float32), (2, 0, 1))], axis=0
    ).reshape(K, 4 * H) * SWT
    W2 = np.concatenate(
        [np.transpose(np.asarray(Uw_i, np.float32), (2, 0, 1)),
         np.transpose(np.asarray(Ww_i, np.float32), (2, 0, 1))], axis=0
    ).reshape(K, 4 * H) * SWT
    wpk = _pack_w(W1, W2)

    br = (np.asarray(Ub_r, np.float32) + np.asarray(Wb_r, np.float32)).reshape(4 * H)
    bi = (np.asarray(Ub_i, np.float32) + np.asarray(Wb_i, np.float32)).reshape(4 * H)
    # bias tile [128, 64]: col = gate*16 + ob*2 + (0 r | 1 i)
    bpk = np.empty((128, 64), np.float32)
    for g in range(4):
        for ob in range(NOB):
            bpk[:, g * 16 + ob * 2 + 0] = br[g * H + ob * 128 : g * H + (ob + 1) * 128]
            bpk[:, g * 16 + ob * 2 + 1] = bi[g * H + ob * 128 : g * H + (ob + 1) * 128]

    X1 = np.concatenate([input[:, :IN], h_x[:, :H]], axis=1) * SX
    X2 = np.concatenate([input[:, IN:], h_x[:, H:]], axis=1) * SX

    in_maps = []
    for c in range(NCORES):
        rows = slice(c * BL, (c + 1) * BL)
        # c_x transposed: [2H, BL] -> [128, 16, BL] (block j = rows j*128+)
        cT = np.ascontiguousarray(
            c_x[rows].T.reshape(16, 128, BL).transpose(1, 0, 2)
        ).astype(np.float16)
        in_maps.append(
            {
                "xpk": _pack_x(X1[rows], X2[rows]),
                "wpk": wpk,
                "ctp": cT,
                "bpk": bpk,
            }
        )

    nc = _get_program()
    res = run_bass_kernel_spmd(
        nc, in_maps, core_ids=list(range(NCORES)), trace=_trace
    )
    h_parts, c_parts = [], []
    for i in range(NCORES):
        hT = res.results[i]["ht_out"].reshape(2 * H, BL)
        cT = res.results[i]["ct_out"].reshape(2 * H, BL)
        h_parts.append(hT.T)
        c_parts.append(cT.T)
    h_t = np.concatenate(h_parts, axis=0).astype(np.float32)
    c_t = np.concatenate(c_parts, axis=0).astype(np.float32)
    if _trace:
        kernel._last_results = res
    return h_t, c_t
